# revision 13
# baseline (speedup 1.0000x reference)
"""Trainium2 Bass kernel for nn_Contour_to_mask (winding-number soft
rasterization of a 128-point contour into a (1, 2, 256, 256) f32 mask).

v8 "three-pass" design. Math: for pixel (i,j) and edge n,
  cross = Pc[n,i] + Qc[n,j],  dot = Rd[n,i] + Sd[n,j]   (separable profiles)
  contribution = s*(pi/2 - arctan(dot/|cross|)) with s = tanh(1e5*cross)
Approximated (rel-err ~1.4e-2 < 2e-2 gate, verified vs reference) as
  contribution = (pi/2)*sgn(cross) - arctan(dot/cross)
so the per-element work collapses to THREE elementwise passes:
  DVE :  one fused 7-stage custom op per image row:
           x = Qc + Pc_i                 (cross, built inline)
           z = BITWISE_NOT(x) * c0'      (fast-reciprocal seed)
           m = z*(1 - x*z)               (1 Newton step; ~0.22% rel err
                                          with c1^2 pre-folded into Sd/Rd)
           q = (Sd'' + Rd''_i) * m       (signed dot/cross, unclipped)
  ACT :  phi = Arctan(q) in f32 (one big call per superblock), plus a
         share of the sign pass as Sign(Qc*1 + Pc_i) (inline bias).
  Pool:  rest of the sign pass as tensor_scalar is_gt: g = (Qc > -Pc_i).
  PE  :  two fp32r one-hot sliding-window reductions over the 128 edges
         into PSUM rows (full-rate at 512-wide blocks): accT = sum(T),
         accP = sum(phi).
  finale (DVE custom): w = min(|accT*C0 + C1 - accP| / 2pi, 1) with
         per-partition C0/C1 selecting the row-pair's sign encoding
         (Pool ge rows: C0=pi, C1=-64pi; ACT Sign rows: C0=pi/2, C1=0).
Host side pre-scales Sd/Rd by c1^2, and nudges Pc by 1 ulp wherever
f32(Qc+Pc) would be exactly/nearly zero (kills the NaN path of the
BITWISE_NOT seed; ~0-2 pixels per run, winding effect < 1e-7).

Sharding: 8 cores; core c handles batch c//4, image rows [(c%4)*64, +64).
"""
import sys

sys.path.insert(0, "/opt/trn_rl_repo")

import numpy as np

SIZE = 256
B = 2
NPTS = 128
N_CORES = 8
ROWS_CORE = SIZE * SIZE * B // (N_CORES * SIZE)  # 64 image rows per core
RPB = 8                        # rows per superblock
SBLK = RPB * SIZE              # 2048 px
NSBLK = ROWS_CORE // RPB       # 8
BLK = 512                      # pixels per reduction block (one PSUM bank)
NBLK = ROWS_CORE * SIZE // BLK  # 32
ROWS_PER_BLK = BLK // SIZE     # 2 image rows per PSUM row

# tuned on a +-[1,2) mantissa grid: max rel err 0.224% for the 1-NR
# BITWISE_NOT reciprocal z*(1-x*z) with output scale C1SQ folded into Sd/Rd
C0P = -0.11853305
C1SQ = 4.00896949

# row-pair sign-pass flavor: "act" -> ACT Tanh (exact soft sign, pi/2
# encoding), "dve" -> DVE is_gt, else Pool is_gt ({0,1}, pi encoding).
# Quotas chosen to equalize engine busy time (ACT ~7 pairs, DVE 2 pairs).
def _pair_kind(p):
    if p % 4 == 1 and p // 4 != 3:
        return "act"          # 7 pairs
    if p in (7, 23):
        return "dve"          # 2 pairs
    return "pool"             # 23 pairs
PAIR_KIND = [_pair_kind(p) for p in range(NBLK)]
K_TANH = 100000.0

_compiled = {}
_ops = {}


def _register_ops():
    if _ops:
        return _ops
    from concourse import dve_ops
    from concourse.dve_spec import (
        Spec, Src0, Src1, C0, C1, C2, One, minn, lower, AluOp, Bin)
    from concourse.dve_uop import DveOpSpec

    def reg(name, spec):
        if name in dve_ops._SUB_OPCODE_FOR_NAME:
            return next(op for op in dve_ops.OPS if op.name == name)
        row = dve_ops._CUSTOM_DVE_ROW_BASE + len(dve_ops.OPS)
        sha = {ver: DveOpSpec(name=name, opcode=row,
                              uops=lower(spec, ver=ver), rd1_en=True).sha(ver)
               for ver in ("v3", "v4")}
        op = dve_ops.DveOp(name, spec, subdim=False, uops_sha=sha)
        dve_ops.OPS.append(op)
        dve_ops.CUSTOM_DVE_SPECS[name] = spec
        dve_ops._SUB_OPCODE_FOR_NAME[name] = row
        return op

    f32 = np.float32

    # FUSEDQ: q = (Src1 + C1) * (z*(1-x*z)), x = Src0 + C0, z = NOT(x)*C2
    _x = Src0 + C0
    _z = Bin(AluOp.BITWISE_NOT, _x, _x) * C2
    _m = _z * (One - _x * _z)
    _body_q = (Src1 + C1) * _m

    def _ref_q(in0, in1, s0, s1, imm2):
        x = (in0 + s0).astype(f32)
        z = ((~x.view(np.int32)).view(f32) * f32(imm2)).astype(f32)
        m = (z * (f32(1.0) - (x * z).astype(f32)).astype(f32)).astype(f32)
        return ((in1 + s1).astype(f32) * m).astype(f32)

    _ops["q"] = reg("FUSEDQ_V8", Spec(body=_body_q, reference=_ref_q))

    # FIN2: w = min(|Src0*C0 + C1 - Src1| * C2, 1)
    _b = Src0 * C0 + C1
    _body_f = minn(Bin(AluOp.ABSOLUTE_DIFF, _b, Src1) * C2, One)

    def _ref_f(in0, in1, s0, s1, imm2):
        return np.minimum(
            np.abs((in0 * s0 + s1) - in1) * f32(imm2), f32(1.0)).astype(f32)

    _ops["fin"] = reg("FIN2_V8", Spec(body=_body_f, reference=_ref_f))
    return _ops


def _build():
    import concourse.bacc as bacc
    import concourse.tile as tile
    import concourse.mybir as mybir

    AF = mybir.ActivationFunctionType
    ALU = mybir.AluOpType
    f32 = mybir.dt.float32
    f32r = mybir.dt.float32r
    ops = _register_ops()

    nc = bacc.Bacc("TRN2", target_bir_lowering=False, debug=False,
                   num_devices=N_CORES)

    # blob = [pc | npc | pck | rd2 | fc0 | fc1 | qc | sd2] along free dim
    BW = 4 * ROWS_CORE + 4 + 2 * SIZE
    blob_d = nc.dram_tensor("blob", [NPTS, BW], f32, kind="ExternalInput").ap()
    NHLF = NBLK // 2
    redg_d = nc.dram_tensor("redg", [NPTS, 2 * NHLF - 1], f32r,
                            kind="ExternalInput").ap()
    out_d = nc.dram_tensor("out", [NBLK, BLK], f32, kind="ExternalOutput").ap()

    with tile.TileContext(nc) as tc:
        with tc.tile_pool(name="cst", bufs=1) as cst, \
             tc.tile_pool(name="work", bufs=3) as work, \
             tc.tile_pool(name="pacc", bufs=1, space="PSUM") as pacc:
            blob_t = cst.tile([NPTS, BW], f32, name="blob_t")
            redg_t = cst.tile([NPTS, 2 * NHLF - 1], f32r, name="redg_t")
            nc.sync.dma_start(blob_t[:], blob_d[:])
            nc.sync.dma_start(redg_t[:], redg_d[:])
            pc_t = blob_t[:, 0 * ROWS_CORE:1 * ROWS_CORE]
            npc_t = blob_t[:, 1 * ROWS_CORE:2 * ROWS_CORE]
            pck_t = blob_t[:, 2 * ROWS_CORE:3 * ROWS_CORE]
            rd2_t = blob_t[:, 3 * ROWS_CORE:4 * ROWS_CORE]
            fc_t = blob_t[:, 4 * ROWS_CORE:4 * ROWS_CORE + 4]
            qc_t = blob_t[:, 4 * ROWS_CORE + 4:4 * ROWS_CORE + 4 + SIZE]
            sd2_t = blob_t[:, 4 * ROWS_CORE + 4 + SIZE:BW]

            # full-bank tiles: matmul PSUM outputs must start at partition 0
            accT = [pacc.tile([NPTS, BLK], f32, name=f"accT{h}")[0:NHLF, :]
                    for h in range(2)]
            accP = [pacc.tile([NPTS, BLK], f32, name=f"accP{h}")[0:NHLF, :]
                    for h in range(2)]

            def emit_front(u):
                """DVE fused-q + the whole sign pass for superblock u."""
                q = work.tile([NPTS, SBLK], f32, tag="q", name=f"q{u}")
                for h in range(RPB):
                    i = u * RPB + h
                    hs = slice(h * SIZE, (h + 1) * SIZE)
                    nc.vector._custom_dve(
                        ops["q"], out=q[:, hs], in0=qc_t, in1=sd2_t,
                        s0=pc_t[:, i:i + 1], s1=rd2_t[:, i:i + 1], imm2=C0P)
                g = work.tile([NPTS, SBLK], f32r, tag="g", name=f"g{u}")
                for h in range(RPB):
                    i = u * RPB + h
                    hs = slice(h * SIZE, (h + 1) * SIZE)
                    kind = PAIR_KIND[i // ROWS_PER_BLK]
                    if kind == "act":
                        nc.scalar.activation(g[:, hs], qc_t, AF.Tanh,
                                             scale=K_TANH,
                                             bias=pck_t[:, i:i + 1])
                    elif kind == "dve":
                        nc.vector.tensor_scalar(g[:, hs], qc_t,
                                                npc_t[:, i:i + 1], None,
                                                ALU.is_gt)
                    else:
                        nc.gpsimd.tensor_scalar(g[:, hs], qc_t,
                                                npc_t[:, i:i + 1], None,
                                                ALU.is_gt)
                # sum(g) matmuls can start as soon as g is written (phi is
                # still in flight) - emit them ahead of the arctan
                for h2 in range(SBLK // BLK):
                    j = (SBLK // BLK) * u + h2
                    hf, jl = j // NHLF, j % NHLF
                    hs = slice(h2 * BLK, (h2 + 1) * BLK)
                    nc.tensor.matmul(accT[hf][:],
                                     redg_t[:, NHLF - 1 - jl:2 * NHLF - 1 - jl],
                                     g[:, hs],
                                     start=(jl == 0), stop=(jl == NHLF - 1))
                return q

            def emit_back(u, q):
                """Arctan + phi reduction for superblock u."""
                phi = work.tile([NPTS, SBLK], f32r, tag="phi", name=f"phi{u}")
                nc.scalar.activation(phi[:], q[:], AF.Arctan)
                for h2 in range(SBLK // BLK):
                    j = (SBLK // BLK) * u + h2
                    hf, jl = j // NHLF, j % NHLF
                    hs = slice(h2 * BLK, (h2 + 1) * BLK)
                    nc.tensor.matmul(accP[hf][:],
                                     redg_t[:, NHLF - 1 - jl:2 * NHLF - 1 - jl],
                                     phi[:, hs],
                                     start=(jl == 0), stop=(jl == NHLF - 1))

            def emit_finale(hf):
                r0, r1 = hf * NHLF, (hf + 1) * NHLF
                pcopy = work.tile([NHLF, BLK], f32, tag=f"pcopy{hf}",
                                  name=f"pcopy{hf}")
                nc.scalar.copy(pcopy[:], accP[hf][:])
                w = work.tile([NHLF, BLK], f32, tag=f"w{hf}", name=f"w{hf}")
                nc.vector._custom_dve(
                    ops["fin"], out=w[:], in0=accT[hf][:], in1=pcopy[:],
                    s0=fc_t[0:NHLF, 2 * hf:2 * hf + 1],
                    s1=fc_t[0:NHLF, 2 * hf + 1:2 * hf + 2],
                    imm2=float(np.float32(1.0 / (2.0 * np.pi))))
                nc.sync.dma_start(out_d[r0:r1, :], w[:])

            prev = None
            for u in range(NSBLK):
                q = emit_front(u)
                if prev is not None:
                    emit_back(*prev)
                    if prev[0] == NSBLK // 2:
                        emit_finale(0)
                prev = (u, q)
            emit_back(*prev)
            emit_finale(1)

    nc.compile()
    return nc


def _host_inputs(contour: np.ndarray):
    """Per-core in_maps from the full (B, NPTS, 2) contour."""
    f32 = np.float32
    mx = (np.arange(SIZE) / SIZE).astype(np.float64)
    my = mx

    prof = []
    for b in range(B):
        cx = contour[b, :, 0].astype(np.float64)
        cy = contour[b, :, 1].astype(np.float64)
        cxn = np.roll(cx, -1)
        cyn = np.roll(cy, -1)
        A = cy * cxn - cx * cyn
        Bc = cyn - cy
        Cc = cx - cxn
        Dd = cx * cxn + cy * cyn
        Ed = -(cx + cxn)
        Fd = -(cy + cyn)
        Pc = (A[:, None] + Bc[:, None] * mx[None, :]).astype(f32)
        Qc = (Cc[:, None] * my[None, :]).astype(f32)
        Rd2 = ((Dd[:, None] + Ed[:, None] * mx[None, :] + mx[None, :] ** 2)
               * C1SQ).astype(f32)
        Sd2 = ((Fd[:, None] * my[None, :] + my[None, :] ** 2) * C1SQ).astype(f32)
        # zero-exterminator: f32(Qc + Pc_i) == +-0 would NaN the NOT seed.
        for _ in range(4):
            cross = Qc[:, None, :] + Pc[:, :, None]  # f32 [N, i, j]
            n_, i_ = np.nonzero((np.abs(cross) < 1e-30).any(axis=2))
            if len(n_) == 0:
                break
            Pc[n_, i_] = np.nextafter(Pc[n_, i_], f32(np.inf), dtype=f32)
        prof.append((Pc, Qc, Rd2, Sd2))

    redg = np.zeros((NPTS, NBLK - 1), dtype=f32)
    redg[:, NBLK // 2 - 1] = 1.0
    is_act = np.array([k == "act" for k in PAIR_KIND])
    fc0 = np.where(is_act, f32(np.pi / 2), f32(np.pi)).astype(f32)[:, None]
    fc1 = np.where(is_act, f32(0.0), f32(-64.0 * np.pi)).astype(f32)[:, None]

    in_maps = []
    for c in range(N_CORES):
        b = c // (N_CORES // B)
        r0 = (c % (N_CORES // B)) * ROWS_CORE
        Pc, Qc, Rd2, Sd2 = prof[b]
        pc = Pc[:, r0:r0 + ROWS_CORE]
        fcpad = np.zeros((NPTS, 4), dtype=f32)
        nh = NBLK // 2
        fcpad[:nh, 0] = fc0[:nh, 0]
        fcpad[:nh, 1] = fc1[:nh, 0]
        fcpad[:nh, 2] = fc0[nh:, 0]
        fcpad[:nh, 3] = fc1[nh:, 0]
        blob = np.concatenate(
            [pc, -pc, (pc.astype(np.float64) * K_TANH).astype(f32),
             Rd2[:, r0:r0 + ROWS_CORE], fcpad, Qc, Sd2], axis=1)
        in_maps.append({
            "blob": np.ascontiguousarray(blob),
            "redg": redg,
        })
    return in_maps


def kernel(contour: np.ndarray) -> np.ndarray:
    from concourse import bass_utils

    contour = np.asarray(contour, dtype=np.float32)
    if "nc" not in _compiled:
        _compiled["nc"] = _build()
    in_maps = _host_inputs(contour)
    res = bass_utils.run_bass_kernel_spmd(
        _compiled["nc"], in_maps, core_ids=list(range(N_CORES))).results

    mask = np.zeros((1, B, SIZE, SIZE), dtype=np.float32)
    for c in range(N_CORES):
        b = c // (N_CORES // B)
        r0 = (c % (N_CORES // B)) * ROWS_CORE
        mask[0, b, r0:r0 + ROWS_CORE, :] = (
            res[c]["out"].reshape(ROWS_CORE, SIZE))
    return mask


# revision 15
# speedup vs baseline: 1.0750x; 1.0750x over previous
"""Trainium2 Bass kernel for nn_Contour_to_mask (winding-number soft
rasterization of a 128-point contour into a (1, 2, 256, 256) f32 mask).

v8 "three-pass" design. Math: for pixel (i,j) and edge n,
  cross = Pc[n,i] + Qc[n,j],  dot = Rd[n,i] + Sd[n,j]   (separable profiles)
  contribution = s*(pi/2 - arctan(dot/|cross|)) with s = tanh(1e5*cross)
Approximated (rel-err ~1.4e-2 < 2e-2 gate, verified vs reference) as
  contribution = (pi/2)*sgn(cross) - arctan(dot/cross)
so the per-element work collapses to THREE elementwise passes:
  DVE :  one fused 7-stage custom op per image row:
           x = Qc + Pc_i                 (cross, built inline)
           z = BITWISE_NOT(x) * c0'      (fast-reciprocal seed)
           m = z*(1 - x*z)               (1 Newton step; ~0.22% rel err
                                          with c1^2 pre-folded into Sd/Rd)
           q = (Sd'' + Rd''_i) * m       (signed dot/cross, unclipped)
  ACT :  phi = Arctan(q) in f32 (one big call per superblock), plus a
         share of the sign pass as Sign(Qc*1 + Pc_i) (inline bias).
  Pool:  rest of the sign pass as tensor_scalar is_gt: g = (Qc > -Pc_i).
  PE  :  two fp32r one-hot sliding-window reductions over the 128 edges
         into PSUM rows (full-rate at 512-wide blocks): accT = sum(T),
         accP = sum(phi).
  finale (DVE custom): w = min(|accT*C0 + C1 - accP| / 2pi, 1) with
         per-partition C0/C1 selecting the row-pair's sign encoding
         (Pool ge rows: C0=pi, C1=-64pi; ACT Sign rows: C0=pi/2, C1=0).
Host side pre-scales Sd/Rd by c1^2, and nudges Pc by 1 ulp wherever
f32(Qc+Pc) would be exactly/nearly zero (kills the NaN path of the
BITWISE_NOT seed; ~0-2 pixels per run, winding effect < 1e-7).

Sharding: 8 cores; core c handles batch c//4, image rows [(c%4)*64, +64).
"""
import sys

sys.path.insert(0, "/opt/trn_rl_repo")

import numpy as np

SIZE = 256
B = 2
NPTS = 128
N_CORES = 8
ROWS_CORE = SIZE * SIZE * B // (N_CORES * SIZE)  # 64 image rows per core
RPB = 8                        # rows per superblock
SBLK = RPB * SIZE              # 2048 px
NSBLK = ROWS_CORE // RPB       # 8
BLK = 512                      # pixels per reduction block (one PSUM bank)
NBLK = ROWS_CORE * SIZE // BLK  # 32
ROWS_PER_BLK = BLK // SIZE     # 2 image rows per PSUM row

# tuned on a +-[1,2) mantissa grid: max rel err 0.224% for the 1-NR
# BITWISE_NOT reciprocal z*(1-x*z) with output scale C1SQ folded into Sd/Rd
C0P = -0.11853305
C1SQ = 4.00896949

# row-pair sign-pass flavor: "act" -> ACT Tanh (exact soft sign, pi/2
# encoding), "dve" -> DVE is_gt, else Pool is_gt ({0,1}, pi encoding).
# Quotas chosen to equalize engine busy time (ACT ~7 pairs, DVE 2 pairs).
def _pair_kind(p):
    if p % 4 == 1 and p // 4 != 3:
        return "act"          # 7 pairs
    if p in (7, 23):
        return "dve"          # 2 pairs
    return "pool"             # 23 pairs
PAIR_KIND = [_pair_kind(p) for p in range(NBLK)]
K_TANH = 100000.0

_compiled = {}
_ops = {}


def _register_ops():
    if _ops:
        return _ops
    from concourse import dve_ops
    from concourse.dve_spec import (
        Spec, Src0, Src1, C0, C1, C2, One, minn, lower, AluOp, Bin)
    from concourse.dve_uop import DveOpSpec

    def reg(name, spec):
        if name in dve_ops._SUB_OPCODE_FOR_NAME:
            return next(op for op in dve_ops.OPS if op.name == name)
        row = dve_ops._CUSTOM_DVE_ROW_BASE + len(dve_ops.OPS)
        sha = {ver: DveOpSpec(name=name, opcode=row,
                              uops=lower(spec, ver=ver), rd1_en=True).sha(ver)
               for ver in ("v3", "v4")}
        op = dve_ops.DveOp(name, spec, subdim=False, uops_sha=sha)
        dve_ops.OPS.append(op)
        dve_ops.CUSTOM_DVE_SPECS[name] = spec
        dve_ops._SUB_OPCODE_FOR_NAME[name] = row
        return op

    f32 = np.float32

    # FUSEDQ: q = (Src1 + C1) * (z*(1-x*z)), x = Src0 + C0, z = NOT(x)*C2
    _x = Src0 + C0
    _z = Bin(AluOp.BITWISE_NOT, _x, _x) * C2
    _m = _z * (One - _x * _z)
    _body_q = (Src1 + C1) * _m

    def _ref_q(in0, in1, s0, s1, imm2):
        x = (in0 + s0).astype(f32)
        z = ((~x.view(np.int32)).view(f32) * f32(imm2)).astype(f32)
        m = (z * (f32(1.0) - (x * z).astype(f32)).astype(f32)).astype(f32)
        return ((in1 + s1).astype(f32) * m).astype(f32)

    _ops["q"] = reg("FUSEDQ_V8", Spec(body=_body_q, reference=_ref_q))

    # FIN2: w = min(|Src0*C0 + C1 - Src1| * C2, 1)
    _b = Src0 * C0 + C1
    _body_f = minn(Bin(AluOp.ABSOLUTE_DIFF, _b, Src1) * C2, One)

    def _ref_f(in0, in1, s0, s1, imm2):
        return np.minimum(
            np.abs((in0 * s0 + s1) - in1) * f32(imm2), f32(1.0)).astype(f32)

    _ops["fin"] = reg("FIN2_V8", Spec(body=_body_f, reference=_ref_f))
    return _ops


def _build():
    import concourse.bacc as bacc
    import concourse.tile as tile
    import concourse.mybir as mybir

    AF = mybir.ActivationFunctionType
    ALU = mybir.AluOpType
    f32 = mybir.dt.float32
    f32r = mybir.dt.float32r
    ops = _register_ops()

    nc = bacc.Bacc("TRN2", target_bir_lowering=False, debug=False,
                   num_devices=N_CORES)

    # blob = [pc | npc | pck | rd2 | fc0 | fc1 | qc | sd2] along free dim
    BW = 4 * ROWS_CORE + 4 + 2 * SIZE
    blob_d = nc.dram_tensor("blob", [NPTS, BW], f32, kind="ExternalInput").ap()
    redg_d = nc.dram_tensor("redg", [NPTS, 2 * NBLK - 1], f32r,
                            kind="ExternalInput").ap()
    out_d = nc.dram_tensor("out", [NBLK, BLK], f32, kind="ExternalOutput").ap()

    with tile.TileContext(nc) as tc:
        with tc.tile_pool(name="cst", bufs=1) as cst, \
             tc.tile_pool(name="work", bufs=3) as work, \
             tc.tile_pool(name="pacc", bufs=1, space="PSUM") as pacc:
            blob_t = cst.tile([NPTS, BW], f32, name="blob_t")
            redg_t = cst.tile([NPTS, 2 * NBLK - 1], f32r, name="redg_t")
            # two parallel HWDGE queues (SP + ACT) halve the input latency
            HB = BW // 2
            nc.sync.dma_start(blob_t[:, 0:HB], blob_d[:, 0:HB])
            nc.scalar.dma_start(blob_t[:, HB:BW], blob_d[:, HB:BW])
            nc.sync.dma_start(redg_t[:], redg_d[:])
            pc_t = blob_t[:, 0 * ROWS_CORE:1 * ROWS_CORE]
            npc_t = blob_t[:, 1 * ROWS_CORE:2 * ROWS_CORE]
            pck_t = blob_t[:, 2 * ROWS_CORE:3 * ROWS_CORE]
            rd2_t = blob_t[:, 3 * ROWS_CORE:4 * ROWS_CORE]
            fc_t = blob_t[:, 4 * ROWS_CORE:4 * ROWS_CORE + 4]
            qc_t = blob_t[:, 4 * ROWS_CORE + 4:4 * ROWS_CORE + 4 + SIZE]
            sd2_t = blob_t[:, 4 * ROWS_CORE + 4 + SIZE:BW]

            accT = pacc.tile([NBLK, BLK], f32, name="accT")
            accP = pacc.tile([NBLK, BLK], f32, name="accP")

            def emit_front(k, r0, nr):
                """DVE fused-q + sign pass + sum(g) matmuls, rows r0..r0+nr."""
                q = work.tile([NPTS, nr * SIZE], f32, tag="q", name=f"q{k}")
                for h in range(nr):
                    i = r0 + h
                    hs = slice(h * SIZE, (h + 1) * SIZE)
                    nc.vector._custom_dve(
                        ops["q"], out=q[:, hs], in0=qc_t, in1=sd2_t,
                        s0=pc_t[:, i:i + 1], s1=rd2_t[:, i:i + 1], imm2=C0P)
                g = work.tile([NPTS, nr * SIZE], f32r, tag="g", name=f"g{k}")
                for h in range(nr):
                    i = r0 + h
                    hs = slice(h * SIZE, (h + 1) * SIZE)
                    kind = PAIR_KIND[i // ROWS_PER_BLK]
                    if kind == "act":
                        nc.scalar.activation(g[:, hs], qc_t, AF.Tanh,
                                             scale=K_TANH,
                                             bias=pck_t[:, i:i + 1])
                    elif kind == "dve":
                        nc.vector.tensor_scalar(g[:, hs], qc_t,
                                                npc_t[:, i:i + 1], None,
                                                ALU.is_gt)
                    else:
                        nc.gpsimd.tensor_scalar(g[:, hs], qc_t,
                                                npc_t[:, i:i + 1], None,
                                                ALU.is_gt)
                for h2 in range(nr // ROWS_PER_BLK):
                    j = r0 // ROWS_PER_BLK + h2
                    hs = slice(h2 * BLK, (h2 + 1) * BLK)
                    nc.tensor.matmul(accT[:],
                                     redg_t[:, NBLK - 1 - j:2 * NBLK - 1 - j],
                                     g[:, hs],
                                     start=(j == 0), stop=(j == NBLK - 1))
                return q

            def emit_back(k, r0, nr, q):
                phi = work.tile([NPTS, nr * SIZE], f32r, tag="phi",
                                name=f"phi{k}")
                nc.scalar.activation(phi[:], q[:], AF.Arctan)
                for h2 in range(nr // ROWS_PER_BLK):
                    j = r0 // ROWS_PER_BLK + h2
                    hs = slice(h2 * BLK, (h2 + 1) * BLK)
                    nc.tensor.matmul(accP[:],
                                     redg_t[:, NBLK - 1 - j:2 * NBLK - 1 - j],
                                     phi[:, hs],
                                     start=(j == 0), stop=(j == NBLK - 1))

            # tapered superblocks: small tail blocks shorten the last
            # arctan -> matmul -> finale critical chain
            SBLKS = [8] * 7 + [4, 2, 2]
            prev = None
            r0 = 0
            for k, nr in enumerate(SBLKS):
                q = emit_front(k, r0, nr)
                if prev is not None:
                    emit_back(*prev)
                prev = (k, r0, nr, q)
                r0 += nr
            emit_back(*prev)

            # finale, pipelined in two column halves
            HC = BLK // 2
            pcopy = work.tile([NBLK, BLK], f32, tag="pcopy", name="pcopy")
            w = work.tile([NBLK, BLK], f32, tag="w", name="w")
            for cs in (slice(0, HC), slice(HC, BLK)):
                nc.scalar.copy(pcopy[:, cs], accP[:, cs])
                nc.vector._custom_dve(
                    ops["fin"], out=w[:, cs], in0=accT[:, cs],
                    in1=pcopy[:, cs],
                    s0=fc_t[0:NBLK, 0:1], s1=fc_t[0:NBLK, 1:2],
                    imm2=float(np.float32(1.0 / (2.0 * np.pi))))
                nc.sync.dma_start(out_d[:, cs], w[:, cs])

    nc.compile()
    return nc


def _host_inputs(contour: np.ndarray):
    """Per-core in_maps from the full (B, NPTS, 2) contour."""
    f32 = np.float32
    mx = (np.arange(SIZE) / SIZE).astype(np.float64)
    my = mx

    prof = []
    for b in range(B):
        cx = contour[b, :, 0].astype(np.float64)
        cy = contour[b, :, 1].astype(np.float64)
        cxn = np.roll(cx, -1)
        cyn = np.roll(cy, -1)
        A = cy * cxn - cx * cyn
        Bc = cyn - cy
        Cc = cx - cxn
        Dd = cx * cxn + cy * cyn
        Ed = -(cx + cxn)
        Fd = -(cy + cyn)
        Pc = (A[:, None] + Bc[:, None] * mx[None, :]).astype(f32)
        Qc = (Cc[:, None] * my[None, :]).astype(f32)
        Rd2 = ((Dd[:, None] + Ed[:, None] * mx[None, :] + mx[None, :] ** 2)
               * C1SQ).astype(f32)
        Sd2 = ((Fd[:, None] * my[None, :] + my[None, :] ** 2) * C1SQ).astype(f32)
        # zero-exterminator: f32(Qc + Pc_i) == +-0 would NaN the NOT seed.
        for _ in range(4):
            cross = Qc[:, None, :] + Pc[:, :, None]  # f32 [N, i, j]
            n_, i_ = np.nonzero((np.abs(cross) < 1e-30).any(axis=2))
            if len(n_) == 0:
                break
            Pc[n_, i_] = np.nextafter(Pc[n_, i_], f32(np.inf), dtype=f32)
        prof.append((Pc, Qc, Rd2, Sd2))

    redg = np.zeros((NPTS, 2 * NBLK - 1), dtype=f32)
    redg[:, NBLK - 1] = 1.0
    is_act = np.array([k == "act" for k in PAIR_KIND])
    fc0 = np.where(is_act, f32(np.pi / 2), f32(np.pi)).astype(f32)[:, None]
    fc1 = np.where(is_act, f32(0.0), f32(-64.0 * np.pi)).astype(f32)[:, None]

    in_maps = []
    for c in range(N_CORES):
        b = c // (N_CORES // B)
        r0 = (c % (N_CORES // B)) * ROWS_CORE
        Pc, Qc, Rd2, Sd2 = prof[b]
        pc = Pc[:, r0:r0 + ROWS_CORE]
        fcpad = np.zeros((NPTS, 4), dtype=f32)
        fcpad[:NBLK, 0] = fc0[:, 0]
        fcpad[:NBLK, 1] = fc1[:, 0]
        blob = np.concatenate(
            [pc, -pc, (pc.astype(np.float64) * K_TANH).astype(f32),
             Rd2[:, r0:r0 + ROWS_CORE], fcpad, Qc, Sd2], axis=1)
        in_maps.append({
            "blob": np.ascontiguousarray(blob),
            "redg": redg,
        })
    return in_maps


def kernel(contour: np.ndarray) -> np.ndarray:
    from concourse import bass_utils

    contour = np.asarray(contour, dtype=np.float32)
    if "nc" not in _compiled:
        _compiled["nc"] = _build()
    in_maps = _host_inputs(contour)
    res = bass_utils.run_bass_kernel_spmd(
        _compiled["nc"], in_maps, core_ids=list(range(N_CORES))).results

    mask = np.zeros((1, B, SIZE, SIZE), dtype=np.float32)
    for c in range(N_CORES):
        b = c // (N_CORES // B)
        r0 = (c % (N_CORES // B)) * ROWS_CORE
        mask[0, b, r0:r0 + ROWS_CORE, :] = (
            res[c]["out"].reshape(ROWS_CORE, SIZE))
    return mask


# revision 18
# speedup vs baseline: 1.0792x; 1.0039x over previous
"""Trainium2 Bass kernel for nn_Contour_to_mask (winding-number soft
rasterization of a 128-point contour into a (1, 2, 256, 256) f32 mask).

v8 "three-pass" design. Math: for pixel (i,j) and edge n,
  cross = Pc[n,i] + Qc[n,j],  dot = Rd[n,i] + Sd[n,j]   (separable profiles)
  contribution = s*(pi/2 - arctan(dot/|cross|)) with s = tanh(1e5*cross)
Approximated (rel-err ~1.4e-2 < 2e-2 gate, verified vs reference) as
  contribution = (pi/2)*sgn(cross) - arctan(dot/cross)
so the per-element work collapses to THREE elementwise passes:
  DVE :  one fused 7-stage custom op per image row:
           x = Qc + Pc_i                 (cross, built inline)
           z = BITWISE_NOT(x) * c0'      (fast-reciprocal seed)
           m = z*(1 - x*z)               (1 Newton step; ~0.22% rel err
                                          with c1^2 pre-folded into Sd/Rd)
           q = (Sd'' + Rd''_i) * m       (signed dot/cross, unclipped)
  ACT :  phi = Arctan(q) in f32 (one big call per superblock), plus a
         share of the sign pass as Sign(Qc*1 + Pc_i) (inline bias).
  Pool:  rest of the sign pass as tensor_scalar is_gt: g = (Qc > -Pc_i).
  PE  :  two fp32r one-hot sliding-window reductions over the 128 edges
         into PSUM rows (full-rate at 512-wide blocks): accT = sum(T),
         accP = sum(phi).
  finale (DVE custom): w = min(|accT*C0 + C1 - accP| / 2pi, 1) with
         per-partition C0/C1 selecting the row-pair's sign encoding
         (Pool ge rows: C0=pi, C1=-64pi; ACT Sign rows: C0=pi/2, C1=0).
Host side pre-scales Sd/Rd by c1^2, and nudges Pc by 1 ulp wherever
f32(Qc+Pc) would be exactly/nearly zero (kills the NaN path of the
BITWISE_NOT seed; ~0-2 pixels per run, winding effect < 1e-7).

Sharding: 8 cores; core c handles batch c//4, image rows [(c%4)*64, +64).
"""
import sys

sys.path.insert(0, "/opt/trn_rl_repo")

import numpy as np

SIZE = 256
B = 2
NPTS = 128
N_CORES = 8
ROWS_CORE = SIZE * SIZE * B // (N_CORES * SIZE)  # 64 image rows per core
RPB = 8                        # rows per superblock
SBLK = RPB * SIZE              # 2048 px
NSBLK = ROWS_CORE // RPB       # 8
BLK = 512                      # pixels per reduction block (one PSUM bank)
NBLK = ROWS_CORE * SIZE // BLK  # 32
ROWS_PER_BLK = BLK // SIZE     # 2 image rows per PSUM row

# tuned on a +-[1,2) mantissa grid: max rel err 0.224% for the 1-NR
# BITWISE_NOT reciprocal z*(1-x*z) with output scale C1SQ folded into Sd/Rd
C0P = -0.11853305
C1SQ = 4.00896949

# row-pair sign-pass flavor: "act" -> ACT Tanh (exact soft sign, pi/2
# encoding), "dve" -> DVE is_gt, else Pool is_gt ({0,1}, pi encoding).
# Quotas chosen to equalize engine busy time (ACT ~7 pairs, DVE 2 pairs).
def _pair_kind(p):
    if p % 4 == 1 and p // 4 != 3:
        return "act"          # 7 pairs
    if p in (7, 23):
        return "dve"          # 2 pairs
    return "pool"             # 23 pairs
PAIR_KIND = [_pair_kind(p) for p in range(NBLK)]
K_TANH = 100000.0

_compiled = {}
_ops = {}


def _register_ops():
    if _ops:
        return _ops
    from concourse import dve_ops
    from concourse.dve_spec import (
        Spec, Src0, Src1, C0, C1, C2, One, minn, lower, AluOp, Bin)
    from concourse.dve_uop import DveOpSpec

    def reg(name, spec):
        if name in dve_ops._SUB_OPCODE_FOR_NAME:
            return next(op for op in dve_ops.OPS if op.name == name)
        row = dve_ops._CUSTOM_DVE_ROW_BASE + len(dve_ops.OPS)
        sha = {ver: DveOpSpec(name=name, opcode=row,
                              uops=lower(spec, ver=ver), rd1_en=True).sha(ver)
               for ver in ("v3", "v4")}
        op = dve_ops.DveOp(name, spec, subdim=False, uops_sha=sha)
        dve_ops.OPS.append(op)
        dve_ops.CUSTOM_DVE_SPECS[name] = spec
        dve_ops._SUB_OPCODE_FOR_NAME[name] = row
        return op

    f32 = np.float32

    # FUSEDQ: q = (Src1 + C1) * (z*(1-x*z)), x = Src0 + C0, z = NOT(x)*C2
    _x = Src0 + C0
    _z = Bin(AluOp.BITWISE_NOT, _x, _x) * C2
    _m = _z * (One - _x * _z)
    _body_q = (Src1 + C1) * _m

    def _ref_q(in0, in1, s0, s1, imm2):
        x = (in0 + s0).astype(f32)
        z = ((~x.view(np.int32)).view(f32) * f32(imm2)).astype(f32)
        m = (z * (f32(1.0) - (x * z).astype(f32)).astype(f32)).astype(f32)
        return ((in1 + s1).astype(f32) * m).astype(f32)

    _ops["q"] = reg("FUSEDQ_V8", Spec(body=_body_q, reference=_ref_q))

    # FIN2: w = min(|Src0*C0 + C1 - Src1| * C2, 1)
    _b = Src0 * C0 + C1
    _body_f = minn(Bin(AluOp.ABSOLUTE_DIFF, _b, Src1) * C2, One)

    def _ref_f(in0, in1, s0, s1, imm2):
        return np.minimum(
            np.abs((in0 * s0 + s1) - in1) * f32(imm2), f32(1.0)).astype(f32)

    _ops["fin"] = reg("FIN2_V8", Spec(body=_body_f, reference=_ref_f))
    return _ops


def _build():
    import concourse.bacc as bacc
    import concourse.tile as tile
    import concourse.mybir as mybir

    AF = mybir.ActivationFunctionType
    ALU = mybir.AluOpType
    f32 = mybir.dt.float32
    f32r = mybir.dt.float32r
    ops = _register_ops()

    nc = bacc.Bacc("TRN2", target_bir_lowering=False, debug=False,
                   num_devices=N_CORES)

    # blob = [pc | npc | pck | rd2 | fc | qc | sd2] along free dim
    RW = 2 * NBLK - 1
    BW = 4 * ROWS_CORE + 4 + 2 * SIZE
    blob_d = nc.dram_tensor("blob", [NPTS, BW], f32, kind="ExternalInput").ap()
    redg_d = nc.dram_tensor("redg", [NPTS, RW], f32r, kind="ExternalInput").ap()
    out_d = nc.dram_tensor("out", [NBLK, BLK], f32, kind="ExternalOutput").ap()

    with tile.TileContext(nc) as tc:
        with tc.tile_pool(name="cst", bufs=1) as cst, \
             tc.tile_pool(name="work", bufs=3) as work, \
             tc.tile_pool(name="pacc", bufs=1, space="PSUM") as pacc:
            blob_t = cst.tile([NPTS, BW], f32, name="blob_t")
            nc.sync.dma_start(blob_t[:], blob_d[:])
            redg_t = cst.tile([NPTS, RW], f32r, name="redg_t")
            nc.sync.dma_start(redg_t[:], redg_d[:])
            pc_t = blob_t[:, 0 * ROWS_CORE:1 * ROWS_CORE]
            npc_t = blob_t[:, 1 * ROWS_CORE:2 * ROWS_CORE]
            pck_t = blob_t[:, 2 * ROWS_CORE:3 * ROWS_CORE]
            rd2_t = blob_t[:, 3 * ROWS_CORE:4 * ROWS_CORE]
            fc_t = blob_t[:, 4 * ROWS_CORE:4 * ROWS_CORE + 4]
            qc_t = blob_t[:, 4 * ROWS_CORE + 4:4 * ROWS_CORE + 4 + SIZE]
            sd2_t = blob_t[:, 4 * ROWS_CORE + 4 + SIZE:BW]

            accT = pacc.tile([NBLK, BLK], f32, name="accT")
            accP = pacc.tile([NBLK, BLK], f32, name="accP")

            def emit_front(k, r0, nr):
                """DVE fused-q + sign pass + sum(g) matmuls, rows r0..r0+nr."""
                q = work.tile([NPTS, nr * SIZE], f32, tag="q", name=f"q{k}")
                for h in range(nr):
                    i = r0 + h
                    hs = slice(h * SIZE, (h + 1) * SIZE)
                    nc.vector._custom_dve(
                        ops["q"], out=q[:, hs], in0=qc_t, in1=sd2_t,
                        s0=pc_t[:, i:i + 1], s1=rd2_t[:, i:i + 1], imm2=C0P)
                g = work.tile([NPTS, nr * SIZE], f32r, tag="g", name=f"g{k}")
                for h in range(nr):
                    i = r0 + h
                    hs = slice(h * SIZE, (h + 1) * SIZE)
                    kind = PAIR_KIND[i // ROWS_PER_BLK]
                    if kind == "act":
                        nc.scalar.activation(g[:, hs], qc_t, AF.Tanh,
                                             scale=K_TANH,
                                             bias=pck_t[:, i:i + 1])
                    elif kind == "dve":
                        nc.vector.tensor_scalar(g[:, hs], qc_t,
                                                npc_t[:, i:i + 1], None,
                                                ALU.is_gt)
                    else:
                        nc.gpsimd.tensor_scalar(g[:, hs], qc_t,
                                                npc_t[:, i:i + 1], None,
                                                ALU.is_gt)
                for h2 in range(nr // ROWS_PER_BLK):
                    j = r0 // ROWS_PER_BLK + h2
                    hs = slice(h2 * BLK, (h2 + 1) * BLK)
                    nc.tensor.matmul(accT[:],
                                     redg_t[:, NBLK - 1 - j:2 * NBLK - 1 - j],
                                     g[:, hs],
                                     start=(j == 0), stop=(j == NBLK - 1))
                return q

            def emit_back(k, r0, nr, q):
                phi = work.tile([NPTS, nr * SIZE], f32r, tag="phi",
                                name=f"phi{k}")
                nc.scalar.activation(phi[:], q[:], AF.Arctan)
                for h2 in range(nr // ROWS_PER_BLK):
                    j = r0 // ROWS_PER_BLK + h2
                    hs = slice(h2 * BLK, (h2 + 1) * BLK)
                    nc.tensor.matmul(accP[:],
                                     redg_t[:, NBLK - 1 - j:2 * NBLK - 1 - j],
                                     phi[:, hs],
                                     start=(j == 0), stop=(j == NBLK - 1))

            # tapered superblocks: small tail blocks shorten the last
            # arctan -> matmul -> finale critical chain
            SBLKS = [8] * 7 + [4, 2, 2]
            prev = None
            r0 = 0
            for k, nr in enumerate(SBLKS):
                q = emit_front(k, r0, nr)
                if prev is not None:
                    emit_back(*prev)
                prev = (k, r0, nr, q)
                r0 += nr
            emit_back(*prev)

            # finale, pipelined in two column halves
            HC = BLK // 2
            pcopy = work.tile([NBLK, BLK], f32, tag="pcopy", name="pcopy")
            w = work.tile([NBLK, BLK], f32, tag="w", name="w")
            for cs in (slice(0, HC), slice(HC, BLK)):
                nc.scalar.copy(pcopy[:, cs], accP[:, cs])
                nc.vector._custom_dve(
                    ops["fin"], out=w[:, cs], in0=accT[:, cs],
                    in1=pcopy[:, cs],
                    s0=fc_t[0:NBLK, 0:1], s1=fc_t[0:NBLK, 1:2],
                    imm2=float(np.float32(1.0 / (2.0 * np.pi))))
            nc.sync.dma_start(out_d[:], w[:])

    nc.compile()
    return nc


def _host_inputs(contour: np.ndarray):
    """Per-core in_maps from the full (B, NPTS, 2) contour."""
    f32 = np.float32
    mx = (np.arange(SIZE) / SIZE).astype(np.float64)
    my = mx

    prof = []
    for b in range(B):
        cx = contour[b, :, 0].astype(np.float64)
        cy = contour[b, :, 1].astype(np.float64)
        cxn = np.roll(cx, -1)
        cyn = np.roll(cy, -1)
        A = cy * cxn - cx * cyn
        Bc = cyn - cy
        Cc = cx - cxn
        Dd = cx * cxn + cy * cyn
        Ed = -(cx + cxn)
        Fd = -(cy + cyn)
        Pc = (A[:, None] + Bc[:, None] * mx[None, :]).astype(f32)
        Qc = (Cc[:, None] * my[None, :]).astype(f32)
        Rd2 = ((Dd[:, None] + Ed[:, None] * mx[None, :] + mx[None, :] ** 2)
               * C1SQ).astype(f32)
        Sd2 = ((Fd[:, None] * my[None, :] + my[None, :] ** 2) * C1SQ).astype(f32)
        # zero-exterminator: f32(Qc + Pc_i) == +-0 would NaN the NOT seed.
        for _ in range(4):
            cross = Qc[:, None, :] + Pc[:, :, None]  # f32 [N, i, j]
            n_, i_ = np.nonzero((np.abs(cross) < 1e-30).any(axis=2))
            if len(n_) == 0:
                break
            Pc[n_, i_] = np.nextafter(Pc[n_, i_], f32(np.inf), dtype=f32)
        prof.append((Pc, Qc, Rd2, Sd2))

    redg = np.zeros((NPTS, 2 * NBLK - 1), dtype=f32)
    redg[:, NBLK - 1] = 1.0
    is_act = np.array([k == "act" for k in PAIR_KIND])
    fc0 = np.where(is_act, f32(np.pi / 2), f32(np.pi)).astype(f32)[:, None]
    fc1 = np.where(is_act, f32(0.0), f32(-64.0 * np.pi)).astype(f32)[:, None]

    in_maps = []
    for c in range(N_CORES):
        b = c // (N_CORES // B)
        r0 = (c % (N_CORES // B)) * ROWS_CORE
        Pc, Qc, Rd2, Sd2 = prof[b]
        pc = Pc[:, r0:r0 + ROWS_CORE]
        fcpad = np.zeros((NPTS, 4), dtype=f32)
        fcpad[:NBLK, 0] = fc0[:, 0]
        fcpad[:NBLK, 1] = fc1[:, 0]
        blob = np.concatenate(
            [pc, -pc, (pc.astype(np.float64) * K_TANH).astype(f32),
             Rd2[:, r0:r0 + ROWS_CORE], fcpad, Qc, Sd2], axis=1)
        in_maps.append({
            "blob": np.ascontiguousarray(blob),
            "redg": redg,
        })
    return in_maps


def kernel(contour: np.ndarray) -> np.ndarray:
    from concourse import bass_utils

    contour = np.asarray(contour, dtype=np.float32)
    if "nc" not in _compiled:
        _compiled["nc"] = _build()
    in_maps = _host_inputs(contour)
    res = bass_utils.run_bass_kernel_spmd(
        _compiled["nc"], in_maps, core_ids=list(range(N_CORES))).results

    mask = np.zeros((1, B, SIZE, SIZE), dtype=np.float32)
    for c in range(N_CORES):
        b = c // (N_CORES // B)
        r0 = (c % (N_CORES // B)) * ROWS_CORE
        mask[0, b, r0:r0 + ROWS_CORE, :] = (
            res[c]["out"].reshape(ROWS_CORE, SIZE))
    return mask
